# revision 10
# baseline (speedup 1.0000x reference)
"""GCN (2-layer) + edge-dot decode on 8 TRN2 NeuronCores.

Math (per GCN layer, with dinv = rsqrt(indeg+1)):
    out[v] = dinv[v] * ( sum_{e: dst=v} hs[src_e] + hs[v] ) + b,  hs = dinv (.) (x @ W)
so no per-edge norm values are needed anywhere; all scaling is per-node.

Layer 2 is commuted past W2:  z = (dinv (.) (A_hat g)) @ W2 + b2,  g = dinv (.) relu(out1).

Device layout: nodes permuted (degree-sorted, core-striped so each core owns a
contiguous 6272-row slice). Aggregation via dma_gather row-gathers + TensorE
identity-matmul accumulation into PSUM; hs/g/z tables AllGathered between
phases. dma_gather indices are SIGNED int16, so every gather uses a frame
centered at row 32768 (idx = row - 32768 covers the whole 50176-row table);
the ucode drops a trailing run of negative indices, so each gather appends one
all-positive pad round pointing at a zero (pad-node) row.
"""

import sys
import numpy as np
from contextlib import ExitStack

sys.path.insert(0, "/opt/trn_rl_repo")

import concourse.bass as bass
import concourse.mybir as mybir
from concourse.bass_utils import run_bass_kernel_spmd
from concourse.tile import TileContext, add_dep_helper
from concourse.masks import make_identity
from concourse.library_config import mlp
from concourse.library_overlay import lower_extended_insts

N, E, L = 50000, 800000, 200000
IN, HID, OUT = 256, 128, 64
C = 8                      # cores
NP = 50176                 # padded node count = 392 blocks of 128
NPC = NP // C              # 6272 nodes per core
BPC = NPC // 128           # 49 blocks per core
FBASE = 32768              # gather frame base row (signed int16 centered)
PADIDX = NP - 1 - FBASE    # pad index -> row 50175 (a zero pad-node row), > 0
CH_MAX = 24                # max rounds per gather chunk (excl. appended pad round)
DEC_CHUNK = 16             # decode chunk rounds

CUSTOM_ISA_OPCODES = {"DMAGatherAnt", "DMAScatterAddAnt"}


def _fix_sync_waits(nc):
    """This container's walrus accepts at most one sync-wait per instruction
    and none on custom ISA ucode ops; hoist extras onto preceding drains."""
    f = nc.m.functions[0]
    for b in f.blocks:
        insts = b.instructions
        i = 0
        while i < len(insts):
            ins = insts[i]
            si = ins.sync_info
            nw = len(si.on_wait) if (si is not None and si.on_wait is not None) else 0
            keep = 0 if str(ins.opcode) in CUSTOM_ISA_OPCODES else 1
            if nw > keep:
                waits = list(si.on_wait)
                hoist, keepw = waits[: nw - keep], waits[nw - keep:]
                for j, w in enumerate(hoist):
                    d = mybir.InstDrain(name=f"{ins.name}-wsplit{j}")
                    d.engine = ins.engine
                    d.sync_info = mybir.SyncInfo(on_wait=[w], on_update=[])
                    insts.insert(i + j, d)
                si.on_wait = keepw
                i += len(hoist)
            i += 1


def _sortedpos(p):
    """final position -> position in the degree-sorted sequence."""
    core = p // NPC
    k = (p % NPC) // 128
    lane = p % 128
    return 128 * (8 * k + core) + lane


def _wrap_idx(flat):
    """[n] int16 -> [128, n//16] wrapped in 16 partitions, replicated x8."""
    n = flat.shape[0]
    arr = np.empty((16, n // 16), dtype=np.int16)
    arr[:, :] = flat.reshape(n // 16, 16).T
    return np.tile(arr, (8, 1))


def _prepare(edge_index, edge_label_index):
    src = np.asarray(edge_index[0], dtype=np.int64)
    dst = np.asarray(edge_index[1], dtype=np.int64)
    la = np.asarray(edge_label_index[0], dtype=np.int64)
    lb = np.asarray(edge_label_index[1], dtype=np.int64)

    deg = np.bincount(dst, minlength=N).astype(np.int64)

    # permutation: degree-sorted, core-striped; 176 zero pad nodes at the tail
    sorted_real = np.argsort(-deg, kind="stable")
    seq = np.full(NP, -1, dtype=np.int64)
    seq[:N] = sorted_real
    final_perm = seq[_sortedpos(np.arange(NP))]   # final position -> orig (-1 pad)
    real_mask = final_perm >= 0
    invpos = np.full(N, -1, dtype=np.int64)
    invpos[final_perm[real_mask]] = np.nonzero(real_mask)[0]
    assert final_perm[NP - 1] == -1

    ps = invpos[src]
    pd = invpos[dst]

    # per-node in-edge ranks (dst-major)
    order = np.argsort(pd, kind="stable")
    pd_s = pd[order]
    ps_s = ps[order]
    newgrp = np.empty(E, dtype=bool)
    newgrp[0] = True
    newgrp[1:] = pd_s[1:] != pd_s[:-1]
    gidx = np.nonzero(newgrp)[0]
    rank = np.arange(E) - gidx[np.cumsum(newgrp) - 1]

    lane = pd_s % 128
    core = pd_s // NPC
    slot = (pd_s % NPC) // 128

    nblocks = NP // 128
    KB = np.zeros(nblocks, dtype=np.int64)
    np.maximum.at(KB, pd_s // 128, rank + 1)
    Khat = np.zeros(BPC, dtype=np.int64)
    for k in range(BPC):
        Khat[k] = int(KB[[c * BPC + k for c in range(C)]].max())
    off = np.zeros(BPC + 1, dtype=np.int64)
    off[1:] = np.cumsum(Khat)

    idxT = np.full((C, off[-1], 128), PADIDX, dtype=np.int16)
    idxT[core, off[slot] + rank, lane] = (ps_s - FBASE).astype(np.int16)

    # chunk schedule per block: rounds split into chunks of <= CH_MAX, each
    # gather = chunk rounds + 1 appended all-pad round (trailing positivity)
    chunks = []   # (k, r0, ch)
    for k in range(BPC):
        r = 0
        while r < int(Khat[k]):
            ch = min(CH_MAX, int(Khat[k]) - r)
            chunks.append((k, r, ch))
            r += ch

    # degp per core [128, BPC]
    degp = np.full(NP, 1e30, dtype=np.float32)
    degp[real_mask] = (deg[final_perm[real_mask]] + 1).astype(np.float32)
    degp_core = degp.reshape(C, BPC, 128).transpose(0, 2, 1).copy()

    # decode tables: natural label order per core, chunks of DEC_CHUNK rounds
    pa = invpos[la]
    pb = invpos[lb]
    LPC = L // C
    LROUNDS = (LPC + 127) // 128
    dec_chunks = []
    r = 0
    while r < LROUNDS:
        ch = min(DEC_CHUNK, LROUNDS - r)
        dec_chunks.append((r, ch))
        r += ch

    padrow = np.full(128, PADIDX, dtype=np.int16)
    idx16 = []
    for c in range(C):
        parts = []
        for (k, r0, ch) in chunks:
            flat = idxT[c, off[k] + r0: off[k] + r0 + ch].reshape(-1)
            parts.append(_wrap_idx(np.concatenate([flat, padrow])))
        fa = np.full(LROUNDS * 128, PADIDX, dtype=np.int16)
        fb = np.full(LROUNDS * 128, PADIDX, dtype=np.int16)
        fa[:LPC] = (pa[c * LPC:(c + 1) * LPC] - FBASE).astype(np.int16)
        fb[:LPC] = (pb[c * LPC:(c + 1) * LPC] - FBASE).astype(np.int16)
        for (r0, ch) in dec_chunks:
            parts.append(_wrap_idx(np.concatenate([fa[r0 * 128:(r0 + ch) * 128], padrow])))
        for (r0, ch) in dec_chunks:
            parts.append(_wrap_idx(np.concatenate([fb[r0 * 128:(r0 + ch) * 128], padrow])))
        idx16.append(np.ascontiguousarray(np.concatenate(parts, axis=1)))

    return dict(
        final_perm=final_perm, invpos=invpos, real_mask=real_mask,
        Khat=Khat, off=off, chunks=chunks, dec_chunks=dec_chunks,
        degp_core=degp_core, idx16=idx16,
    )


def _build(prep):
    chunks = prep["chunks"]
    dec_chunks = prep["dec_chunks"]
    TOTW = prep["idx16"][0].shape[1]
    ndec_cols = sum(ch for (_, ch) in dec_chunks)

    nc = bass.Bass(num_devices=C, dynamic_dma_scratch_size=32768, num_swdge_queues=2)
    f32 = mybir.dt.float32
    xT_d = nc.dram_tensor("xT", [IN, NPC], f32, kind="ExternalInput")
    W1_d = nc.dram_tensor("W1", [IN, HID], f32, kind="ExternalInput")
    b1_d = nc.dram_tensor("b1", [1, HID], f32, kind="ExternalInput")
    W2_d = nc.dram_tensor("W2", [HID, OUT], f32, kind="ExternalInput")
    b2_d = nc.dram_tensor("b2", [1, OUT], f32, kind="ExternalInput")
    degp_d = nc.dram_tensor("degp", [128, BPC], f32, kind="ExternalInput")
    idx_d = nc.dram_tensor("idx16", [128, TOTW], mybir.dt.int16, kind="ExternalInput")
    out_d = nc.dram_tensor("out", [128, ndec_cols], f32, kind="ExternalOutput")

    ag1_in = nc.dram_tensor("ag1_in", [NPC, HID], f32)
    tab1 = nc.dram_tensor("tab1", [NP, HID], f32)
    ag2_in = nc.dram_tensor("ag2_in", [NPC, HID], f32)
    tab2 = nc.dram_tensor("tab2", [NP, HID], f32)
    ag3_in = nc.dram_tensor("ag3_in", [NPC, OUT], f32)
    ztab = nc.dram_tensor("ztab", [NP, OUT], f32)

    with TileContext(nc) as tc, ExitStack() as ctx:
        const = ctx.enter_context(tc.tile_pool(name="const", bufs=1))
        own = ctx.enter_context(tc.tile_pool(name="own", bufs=1))
        lp = ctx.enter_context(tc.tile_pool(name="lhsT", bufs=4))
        gp = ctx.enter_context(tc.tile_pool(name="gath", bufs=4))
        dgp = ctx.enter_context(tc.tile_pool(name="dgath", bufs=3))
        pp = ctx.enter_context(tc.tile_pool(name="psA", bufs=3, space="PSUM"))
        pz = ctx.enter_context(tc.tile_pool(name="psZ", bufs=2, space="PSUM"))
        sp_ = ctx.enter_context(tc.tile_pool(name="stage", bufs=4))

        ll = nc.gpsimd.load_library(mlp)

        ident = const.tile([128, 128], f32)
        make_identity(nc, ident[:])

        idx_sb = const.tile([128, TOTW], mybir.dt.int16)
        idma = nc.sync.dma_start(out=idx_sb[:], in_=idx_d[:, :])
        add_dep_helper(idma.ins, ll.ins, reason="idx after lib load")

        kvals = sorted({(ch + 1) * 128 for (_, _, ch) in chunks}
                       | {(ch + 1) * 128 for (_, ch) in dec_chunks})
        kreg = {}
        for v in kvals:
            r = ctx.enter_context(nc.gpsimd.register(f"nidx{v}"))
            nc.gpsimd.reg_mov(r, v)
            kreg[v] = r

        W1_sb = []
        for i in range(2):
            w1t = const.tile([128, HID], f32, tag=f"w1_{i}", name=f"w1_{i}")
            nc.sync.dma_start(out=w1t[:], in_=W1_d[i * 128:(i + 1) * 128, :])
            W1_sb.append(w1t)
        W2_sb = const.tile([128, OUT], f32)
        nc.sync.dma_start(out=W2_sb[:], in_=W2_d[:, :])

        ones_row = const.tile([1, 128], f32)
        nc.vector.memset(ones_row[:], 1.0)
        b1_row = const.tile([1, HID], f32)
        nc.sync.dma_start(out=b1_row[:], in_=b1_d[:, :])
        b2_row = const.tile([1, OUT], f32)
        nc.sync.dma_start(out=b2_row[:], in_=b2_d[:, :])
        bias1 = const.tile([128, HID], f32)
        bps = pz.tile([128, HID], f32, tag="qt")
        nc.tensor.matmul(out=bps[:], lhsT=ones_row[:], rhs=b1_row[:], start=True, stop=True)
        nc.scalar.activation(out=bias1[:], in_=bps[:], func=mybir.ActivationFunctionType.Copy)
        bias2 = const.tile([128, OUT], f32)
        bps2 = pz.tile([128, OUT], f32, tag="qt")
        nc.tensor.matmul(out=bps2[:], lhsT=ones_row[:], rhs=b2_row[:], start=True, stop=True)
        nc.scalar.activation(out=bias2[:], in_=bps2[:], func=mybir.ActivationFunctionType.Copy)

        degp_sb = const.tile([128, BPC], f32)
        nc.sync.dma_start(out=degp_sb[:], in_=degp_d[:, :])
        rec = const.tile([128, BPC], f32)
        nc.vector.reciprocal(out=rec[:], in_=degp_sb[:])
        dinv = const.tile([128, BPC], f32)
        nc.scalar.activation(out=dinv[:], in_=rec[:], func=mybir.ActivationFunctionType.Sqrt)

        hs_own = own.tile([128, NPC], f32)
        g_own = own.tile([128, NPC], f32)

        # chunk -> idx column offsets
        blk_chunks = [[] for _ in range(BPC)]
        co = 0
        for (k, r0, ch) in chunks:
            blk_chunks[k].append((co, ch))
            co += (ch + 1) * 8
        dec_acoffs = []
        for (r0, ch) in dec_chunks:
            dec_acoffs.append(co)
            co += (ch + 1) * 8
        dec_bcoffs = []
        for (r0, ch) in dec_chunks:
            dec_bcoffs.append(co)
            co += (ch + 1) * 8
        assert co == TOTW

        # ---------------- Phase A: GEMM1 -> hs1 ----------------
        wr1 = []
        with nc.named_scope("gemm1"):
            for k in range(BPC):
                ps = pp.tile([128, HID], f32, tag="main", name="psg")
                for i in range(2):
                    lt = lp.tile([128, 128], f32, tag="lhsT", name="lt")
                    nc.sync.dma_start(out=lt[:], in_=xT_d[i * 128:(i + 1) * 128, k * 128:(k + 1) * 128])
                    nc.tensor.matmul(out=ps[:], lhsT=lt[:], rhs=W1_sb[i][:],
                                     start=(i == 0), stop=(i == 1))
                nc.scalar.activation(out=hs_own[:, k * 128:(k + 1) * 128], in_=ps[:],
                                     func=mybir.ActivationFunctionType.Copy,
                                     scale=dinv[:, k:k + 1])
                wr1.append(nc.sync.dma_start(out=ag1_in[k * 128:(k + 1) * 128, :],
                                             in_=hs_own[:, k * 128:(k + 1) * 128]))

        with nc.named_scope("ag1"):
            cc1 = nc.gpsimd.collective_compute(
                "AllGather", mybir.AluOpType.bypass,
                replica_groups=[list(range(C))],
                ins=[ag1_in.ap().opt()], outs=[tab1.ap().opt()],
            )
            for w in wr1:
                add_dep_helper(cc1.ins, w.ins, reason="AG1 after hs writes")

        qctr = [0]

        def aggregate(k, tab, own_tile, cc):
            ps = pp.tile([128, HID], f32, tag="main", name="psagg")
            first = True
            for (coff, ch) in blk_chunks[k]:
                gt = gp.tile([128, ch + 1, HID], f32, tag="gt", name="gt")
                qctr[0] ^= 1
                gi = nc.gpsimd.dma_gather(
                    gt[:], tab[FBASE:, :], idx_sb[:, coff:coff + (ch + 1) * 8],
                    (ch + 1) * 128, kreg[(ch + 1) * 128], HID, single_packet=False,
                    queue_num=qctr[0])
                add_dep_helper(gi.ins, cc.ins, reason="gather after AG")
                for r in range(ch):
                    nc.tensor.matmul(out=ps[:], lhsT=ident[:], rhs=gt[:, r, :],
                                     start=first, stop=False)
                    first = False
            nc.tensor.matmul(out=ps[:], lhsT=ident[:],
                             rhs=own_tile[:, k * 128:(k + 1) * 128],
                             start=first, stop=True)
            return ps

        # ---------------- Phase B: layer-1 aggregation -> g ----------------
        wr2 = []
        with nc.named_scope("agg1"):
            for k in range(BPC):
                ps = aggregate(k, tab1, hs_own, cc1)
                t1 = sp_.tile([128, HID], f32, tag="t1", name="t1")
                nc.scalar.activation(out=t1[:], in_=ps[:],
                                     func=mybir.ActivationFunctionType.Copy,
                                     scale=dinv[:, k:k + 1])
                t2 = sp_.tile([128, HID], f32, tag="t2", name="t2")
                nc.vector.tensor_add(out=t2[:], in0=t1[:], in1=bias1[:])
                nc.scalar.activation(out=g_own[:, k * 128:(k + 1) * 128], in_=t2[:],
                                     func=mybir.ActivationFunctionType.Relu,
                                     scale=dinv[:, k:k + 1])
                wr2.append(nc.sync.dma_start(out=ag2_in[k * 128:(k + 1) * 128, :],
                                             in_=g_own[:, k * 128:(k + 1) * 128]))

        with nc.named_scope("ag2"):
            cc2 = nc.gpsimd.collective_compute(
                "AllGather", mybir.AluOpType.bypass,
                replica_groups=[list(range(C))],
                ins=[ag2_in.ap().opt()], outs=[tab2.ap().opt()],
            )
            for w in wr2:
                add_dep_helper(cc2.ins, w.ins, reason="AG2 after g writes")

        # ---------------- Phase C: layer-2 aggregation + GEMM2 -> z ----------------
        wr3 = []
        with nc.named_scope("agg2"):
            for k in range(BPC):
                ps = aggregate(k, tab2, g_own, cc2)
                q = sp_.tile([128, HID], f32, tag="q", name="q")
                nc.scalar.activation(out=q[:], in_=ps[:],
                                     func=mybir.ActivationFunctionType.Copy,
                                     scale=dinv[:, k:k + 1])
                qt_ps = pz.tile([128, HID], f32, tag="qt", name="qtps")
                nc.tensor.transpose(out=qt_ps[:], in_=q[:], identity=ident[:])
                qt = sp_.tile([128, HID], f32, tag="qt_sb", name="qtsb")
                nc.vector.tensor_copy(out=qt[:], in_=qt_ps[:])
                zps = pz.tile([128, OUT], f32, tag="z", name="zps")
                nc.tensor.matmul(out=zps[:], lhsT=qt[:], rhs=W2_sb[:], start=True, stop=True)
                z = sp_.tile([128, OUT], f32, tag="zsb", name="zsb")
                nc.vector.tensor_add(out=z[:], in0=zps[:], in1=bias2[:])
                wr3.append(nc.sync.dma_start(out=ag3_in[k * 128:(k + 1) * 128, :], in_=z[:]))

        with nc.named_scope("ag3"):
            cc3 = nc.gpsimd.collective_compute(
                "AllGather", mybir.AluOpType.bypass,
                replica_groups=[list(range(C))],
                ins=[ag3_in.ap().opt()], outs=[ztab.ap().opt()],
            )
            for w in wr3:
                add_dep_helper(cc3.ins, w.ins, reason="AG3 after z writes")

        # ---------------- Phase D: decode ----------------
        with nc.named_scope("decode"):
            out_sb = own.tile([128, ndec_cols], f32)
            col = 0
            for i, (r0, ch) in enumerate(dec_chunks):
                za = dgp.tile([128, ch + 1, OUT], f32, tag="za", name="za")
                ga = nc.gpsimd.dma_gather(
                    za[:], ztab[FBASE:, :], idx_sb[:, dec_acoffs[i]:dec_acoffs[i] + (ch + 1) * 8],
                    (ch + 1) * 128, kreg[(ch + 1) * 128], OUT, single_packet=False,
                    queue_num=0)
                add_dep_helper(ga.ins, cc3.ins, reason="decode a after AG3")
                zb = dgp.tile([128, ch + 1, OUT], f32, tag="zb", name="zb")
                gb = nc.gpsimd.dma_gather(
                    zb[:], ztab[FBASE:, :], idx_sb[:, dec_bcoffs[i]:dec_bcoffs[i] + (ch + 1) * 8],
                    (ch + 1) * 128, kreg[(ch + 1) * 128], OUT, single_packet=False,
                    queue_num=1)
                add_dep_helper(gb.ins, cc3.ins, reason="decode b after AG3")
                prod = sp_.tile([128, ch * OUT], f32, tag="prod", name="prod")
                nc.vector.tensor_mul(out=prod[:].rearrange("p (c o) -> p c o", o=OUT),
                                     in0=za[:, :ch, :], in1=zb[:, :ch, :])
                nc.vector.reduce_sum(out=out_sb[:, col:col + ch],
                                     in_=prod[:].rearrange("p (c o) -> p c o", o=OUT),
                                     axis=mybir.AxisListType.X)
                col += ch
            nc.sync.dma_start(out=out_d[:, :], in_=out_sb[:])

    lower_extended_insts(nc)
    _fix_sync_waits(nc)
    return nc


def kernel(x, W1, b1, W2, b2, edge_index, edge_label_index):
    x = np.asarray(x, dtype=np.float32)
    W1 = np.asarray(W1, dtype=np.float32)
    b1 = np.asarray(b1, dtype=np.float32)
    W2 = np.asarray(W2, dtype=np.float32)
    b2 = np.asarray(b2, dtype=np.float32)
    prep = _prepare(np.asarray(edge_index), np.asarray(edge_label_index))
    nc = _build(prep)

    xp = np.zeros((NP, IN), dtype=np.float32)
    rm = prep["real_mask"]
    xp[rm] = x[prep["final_perm"][rm]]

    in_maps = []
    for c in range(C):
        in_maps.append({
            "xT": np.ascontiguousarray(xp[c * NPC:(c + 1) * NPC].T),
            "W1": W1, "b1": b1.reshape(1, HID),
            "W2": W2, "b2": b2.reshape(1, OUT),
            "degp": prep["degp_core"][c],
            "idx16": prep["idx16"][c],
        })
    res = run_bass_kernel_spmd(nc, in_maps, core_ids=list(range(C)))

    LPC = L // C
    out = np.empty(L, dtype=np.float32)
    for c in range(C):
        o = res.results[c]["out"]          # [128, ncols]; label j at (j%128, j//128)
        j = np.arange(LPC)
        out[c * LPC:(c + 1) * LPC] = o[j % 128, j // 128]
    return out


# revision 11
# speedup vs baseline: 1.8821x; 1.8821x over previous
"""GCN (2-layer) + edge-dot decode on 8 TRN2 NeuronCores.

Math (per GCN layer, with dinv = rsqrt(indeg+1)):
    out[v] = dinv[v] * ( sum_{e: dst=v} hs[src_e] + hs[v] ) + b,  hs = dinv (.) (x @ W)
so no per-edge norm values are needed anywhere; all scaling is per-node.

Layer 2 is commuted past W2:  z = (dinv (.) (A_hat g)) @ W2 + b2,  g = dinv (.) relu(out1).

Device layout: nodes permuted (degree-sorted, core-striped so each core owns a
contiguous 6272-row slice). Aggregation via dma_gather row-gathers + TensorE
identity-matmul accumulation into PSUM; hs/g/z tables AllGathered between
phases. dma_gather indices are SIGNED int16, so every gather uses a frame
centered at row 32768 (idx = row - 32768 covers the whole 50176-row table);
the ucode drops a trailing run of negative indices, so each gather appends one
all-positive pad round pointing at a zero (pad-node) row.
"""

import sys
import numpy as np
from contextlib import ExitStack

sys.path.insert(0, "/opt/trn_rl_repo")

import concourse.bass as bass
import concourse.mybir as mybir
from concourse.bass_utils import run_bass_kernel_spmd
from concourse.tile import TileContext, add_dep_helper
from concourse.masks import make_identity
from concourse.library_config import mlp
from concourse.library_overlay import lower_extended_insts

N, E, L = 50000, 800000, 200000
IN, HID, OUT = 256, 128, 64
C = 8                      # cores
NP = 50176                 # padded node count = 392 blocks of 128
NPC = NP // C              # 6272 nodes per core
BPC = NPC // 128           # 49 blocks per core
FBASE = 32768              # gather frame base row (signed int16 centered)
PADIDX = NP - 1 - FBASE    # pad index -> row 50175 (a zero pad-node row), > 0
CH_MAX = 24                # max rounds per gather chunk (excl. appended pad round)
DEC_CHUNK = 16             # decode chunk rounds

CUSTOM_ISA_OPCODES = {"DMAGatherAnt", "DMAScatterAddAnt"}


def _fix_sync_waits(nc):
    """This container's walrus accepts at most one sync-wait per instruction
    and none on custom ISA ucode ops; hoist extras onto preceding drains."""
    f = nc.m.functions[0]
    for b in f.blocks:
        insts = b.instructions
        i = 0
        while i < len(insts):
            ins = insts[i]
            si = ins.sync_info
            nw = len(si.on_wait) if (si is not None and si.on_wait is not None) else 0
            keep = 0 if str(ins.opcode) in CUSTOM_ISA_OPCODES else 1
            if nw > keep:
                waits = list(si.on_wait)
                hoist, keepw = waits[: nw - keep], waits[nw - keep:]
                for j, w in enumerate(hoist):
                    d = mybir.InstEventSemaphore(name=f"{ins.name}-wsplit{j}")
                    d.engine = ins.engine
                    d.sync_info = mybir.SyncInfo(on_wait=[w], on_update=[])
                    insts.insert(i + j, d)
                si.on_wait = keepw
                i += len(hoist)
            i += 1


def _sortedpos(p):
    """final position -> position in the degree-sorted sequence."""
    core = p // NPC
    k = (p % NPC) // 128
    lane = p % 128
    return 128 * (8 * k + core) + lane


def _wrap_idx(flat):
    """[n] int16 -> [128, n//16] wrapped in 16 partitions, replicated x8."""
    n = flat.shape[0]
    arr = np.empty((16, n // 16), dtype=np.int16)
    arr[:, :] = flat.reshape(n // 16, 16).T
    return np.tile(arr, (8, 1))


def _prepare(edge_index, edge_label_index):
    src = np.asarray(edge_index[0], dtype=np.int64)
    dst = np.asarray(edge_index[1], dtype=np.int64)
    la = np.asarray(edge_label_index[0], dtype=np.int64)
    lb = np.asarray(edge_label_index[1], dtype=np.int64)

    deg = np.bincount(dst, minlength=N).astype(np.int64)

    # permutation: degree-sorted, core-striped; 176 zero pad nodes at the tail
    sorted_real = np.argsort(-deg, kind="stable")
    seq = np.full(NP, -1, dtype=np.int64)
    seq[:N] = sorted_real
    final_perm = seq[_sortedpos(np.arange(NP))]   # final position -> orig (-1 pad)
    real_mask = final_perm >= 0
    invpos = np.full(N, -1, dtype=np.int64)
    invpos[final_perm[real_mask]] = np.nonzero(real_mask)[0]
    assert final_perm[NP - 1] == -1

    ps = invpos[src]
    pd = invpos[dst]

    # per-node in-edge ranks (dst-major)
    order = np.argsort(pd, kind="stable")
    pd_s = pd[order]
    ps_s = ps[order]
    newgrp = np.empty(E, dtype=bool)
    newgrp[0] = True
    newgrp[1:] = pd_s[1:] != pd_s[:-1]
    gidx = np.nonzero(newgrp)[0]
    rank = np.arange(E) - gidx[np.cumsum(newgrp) - 1]

    lane = pd_s % 128
    core = pd_s // NPC
    slot = (pd_s % NPC) // 128

    nblocks = NP // 128
    KB = np.zeros(nblocks, dtype=np.int64)
    np.maximum.at(KB, pd_s // 128, rank + 1)
    Khat = np.zeros(BPC, dtype=np.int64)
    for k in range(BPC):
        Khat[k] = int(KB[[c * BPC + k for c in range(C)]].max())
    off = np.zeros(BPC + 1, dtype=np.int64)
    off[1:] = np.cumsum(Khat)

    idxT = np.full((C, off[-1], 128), PADIDX, dtype=np.int16)
    idxT[core, off[slot] + rank, lane] = (ps_s - FBASE).astype(np.int16)

    # chunk schedule per block: rounds split into chunks of <= CH_MAX, each
    # gather = chunk rounds + 1 appended all-pad round (trailing positivity)
    chunks = []   # (k, r0, ch)
    for k in range(BPC):
        r = 0
        while r < int(Khat[k]):
            ch = min(CH_MAX, int(Khat[k]) - r)
            chunks.append((k, r, ch))
            r += ch

    # degp per core [128, BPC]
    degp = np.full(NP, 1e30, dtype=np.float32)
    degp[real_mask] = (deg[final_perm[real_mask]] + 1).astype(np.float32)
    degp_core = degp.reshape(C, BPC, 128).transpose(0, 2, 1).copy()

    # decode tables: natural label order per core, chunks of DEC_CHUNK rounds
    pa = invpos[la]
    pb = invpos[lb]
    LPC = L // C
    LROUNDS = (LPC + 127) // 128
    dec_chunks = []
    r = 0
    while r < LROUNDS:
        ch = min(DEC_CHUNK, LROUNDS - r)
        dec_chunks.append((r, ch))
        r += ch

    padrow = np.full(128, PADIDX, dtype=np.int16)
    idx16 = []
    for c in range(C):
        parts = []
        for (k, r0, ch) in chunks:
            flat = idxT[c, off[k] + r0: off[k] + r0 + ch].reshape(-1)
            parts.append(_wrap_idx(np.concatenate([flat, padrow])))
        fa = np.full(LROUNDS * 128, PADIDX, dtype=np.int16)
        fb = np.full(LROUNDS * 128, PADIDX, dtype=np.int16)
        fa[:LPC] = (pa[c * LPC:(c + 1) * LPC] - FBASE).astype(np.int16)
        fb[:LPC] = (pb[c * LPC:(c + 1) * LPC] - FBASE).astype(np.int16)
        for (r0, ch) in dec_chunks:
            parts.append(_wrap_idx(np.concatenate([fa[r0 * 128:(r0 + ch) * 128], padrow])))
        for (r0, ch) in dec_chunks:
            parts.append(_wrap_idx(np.concatenate([fb[r0 * 128:(r0 + ch) * 128], padrow])))
        idx16.append(np.ascontiguousarray(np.concatenate(parts, axis=1)))

    return dict(
        final_perm=final_perm, invpos=invpos, real_mask=real_mask,
        Khat=Khat, off=off, chunks=chunks, dec_chunks=dec_chunks,
        degp_core=degp_core, idx16=idx16,
    )


def _build(prep):
    chunks = prep["chunks"]
    dec_chunks = prep["dec_chunks"]
    TOTW = prep["idx16"][0].shape[1]
    ndec_cols = sum(ch for (_, ch) in dec_chunks)

    nc = bass.Bass(num_devices=C, dynamic_dma_scratch_size=32768, num_swdge_queues=2)
    f32 = mybir.dt.float32
    xT_d = nc.dram_tensor("xT", [IN, NPC], f32, kind="ExternalInput")
    W1_d = nc.dram_tensor("W1", [IN, HID], f32, kind="ExternalInput")
    b1_d = nc.dram_tensor("b1", [1, HID], f32, kind="ExternalInput")
    W2_d = nc.dram_tensor("W2", [HID, OUT], f32, kind="ExternalInput")
    b2_d = nc.dram_tensor("b2", [1, OUT], f32, kind="ExternalInput")
    degp_d = nc.dram_tensor("degp", [128, BPC], f32, kind="ExternalInput")
    idx_d = nc.dram_tensor("idx16", [128, TOTW], mybir.dt.int16, kind="ExternalInput")
    out_d = nc.dram_tensor("out", [128, ndec_cols], f32, kind="ExternalOutput")

    ag1_in = nc.dram_tensor("ag1_in", [NPC, HID], f32)
    tab1 = nc.dram_tensor("tab1", [NP, HID], f32)
    ag2_in = nc.dram_tensor("ag2_in", [NPC, HID], f32)
    tab2 = nc.dram_tensor("tab2", [NP, HID], f32)
    ag3_in = nc.dram_tensor("ag3_in", [NPC, OUT], f32)
    ztab = nc.dram_tensor("ztab", [NP, OUT], f32)

    with TileContext(nc) as tc, ExitStack() as ctx:
        const = ctx.enter_context(tc.tile_pool(name="const", bufs=1))
        own = ctx.enter_context(tc.tile_pool(name="own", bufs=1))
        lp = ctx.enter_context(tc.tile_pool(name="lhsT", bufs=4))
        gp = ctx.enter_context(tc.tile_pool(name="gath", bufs=4))
        dgp = ctx.enter_context(tc.tile_pool(name="dgath", bufs=3))
        pp = ctx.enter_context(tc.tile_pool(name="psA", bufs=3, space="PSUM"))
        pz = ctx.enter_context(tc.tile_pool(name="psZ", bufs=2, space="PSUM"))
        sp_ = ctx.enter_context(tc.tile_pool(name="stage", bufs=4))

        ll = nc.gpsimd.load_library(mlp)

        ident = const.tile([128, 128], f32)
        make_identity(nc, ident[:])

        idx_sb = const.tile([128, TOTW], mybir.dt.int16)
        idma = nc.sync.dma_start(out=idx_sb[:], in_=idx_d[:, :])
        add_dep_helper(idma.ins, ll.ins, reason="idx after lib load")

        kvals = sorted({(ch + 1) * 128 for (_, _, ch) in chunks}
                       | {(ch + 1) * 128 for (_, ch) in dec_chunks})
        kreg = {}
        for v in kvals:
            r = ctx.enter_context(nc.gpsimd.register(f"nidx{v}"))
            nc.gpsimd.reg_mov(r, v)
            kreg[v] = r

        W1_sb = []
        for i in range(2):
            w1t = const.tile([128, HID], f32, tag=f"w1_{i}", name=f"w1_{i}")
            nc.sync.dma_start(out=w1t[:], in_=W1_d[i * 128:(i + 1) * 128, :])
            W1_sb.append(w1t)
        W2_sb = const.tile([128, OUT], f32)
        nc.sync.dma_start(out=W2_sb[:], in_=W2_d[:, :])

        ones_row = const.tile([1, 128], f32)
        nc.vector.memset(ones_row[:], 1.0)
        b1_row = const.tile([1, HID], f32)
        nc.sync.dma_start(out=b1_row[:], in_=b1_d[:, :])
        b2_row = const.tile([1, OUT], f32)
        nc.sync.dma_start(out=b2_row[:], in_=b2_d[:, :])
        bias1 = const.tile([128, HID], f32)
        bps = pz.tile([128, HID], f32, tag="qt")
        nc.tensor.matmul(out=bps[:], lhsT=ones_row[:], rhs=b1_row[:], start=True, stop=True)
        nc.scalar.activation(out=bias1[:], in_=bps[:], func=mybir.ActivationFunctionType.Copy)
        bias2 = const.tile([128, OUT], f32)
        bps2 = pz.tile([128, OUT], f32, tag="qt")
        nc.tensor.matmul(out=bps2[:], lhsT=ones_row[:], rhs=b2_row[:], start=True, stop=True)
        nc.scalar.activation(out=bias2[:], in_=bps2[:], func=mybir.ActivationFunctionType.Copy)

        degp_sb = const.tile([128, BPC], f32)
        nc.sync.dma_start(out=degp_sb[:], in_=degp_d[:, :])
        rec = const.tile([128, BPC], f32)
        nc.vector.reciprocal(out=rec[:], in_=degp_sb[:])
        dinv = const.tile([128, BPC], f32)
        nc.scalar.activation(out=dinv[:], in_=rec[:], func=mybir.ActivationFunctionType.Sqrt)

        hs_own = own.tile([128, NPC], f32)
        g_own = own.tile([128, NPC], f32)

        # chunk -> idx column offsets
        blk_chunks = [[] for _ in range(BPC)]
        co = 0
        for (k, r0, ch) in chunks:
            blk_chunks[k].append((co, ch))
            co += (ch + 1) * 8
        dec_acoffs = []
        for (r0, ch) in dec_chunks:
            dec_acoffs.append(co)
            co += (ch + 1) * 8
        dec_bcoffs = []
        for (r0, ch) in dec_chunks:
            dec_bcoffs.append(co)
            co += (ch + 1) * 8
        assert co == TOTW

        # ---------------- Phase A: GEMM1 -> hs1 ----------------
        wr1 = []
        with nc.named_scope("gemm1"):
            for k in range(BPC):
                ps = pp.tile([128, HID], f32, tag="main", name="psg")
                for i in range(2):
                    lt = lp.tile([128, 128], f32, tag="lhsT", name="lt")
                    nc.sync.dma_start(out=lt[:], in_=xT_d[i * 128:(i + 1) * 128, k * 128:(k + 1) * 128])
                    nc.tensor.matmul(out=ps[:], lhsT=lt[:], rhs=W1_sb[i][:],
                                     start=(i == 0), stop=(i == 1))
                nc.scalar.activation(out=hs_own[:, k * 128:(k + 1) * 128], in_=ps[:],
                                     func=mybir.ActivationFunctionType.Copy,
                                     scale=dinv[:, k:k + 1])
                wr1.append(nc.sync.dma_start(out=ag1_in[k * 128:(k + 1) * 128, :],
                                             in_=hs_own[:, k * 128:(k + 1) * 128]))

        with nc.named_scope("ag1"):
            cc1 = nc.gpsimd.collective_compute(
                "AllGather", mybir.AluOpType.bypass,
                replica_groups=[list(range(C))],
                ins=[ag1_in.ap().opt()], outs=[tab1.ap().opt()],
            )
            for w in wr1:
                add_dep_helper(cc1.ins, w.ins, reason="AG1 after hs writes")

        qctr = [0]

        def aggregate(k, tab, own_tile, cc):
            ps = pp.tile([128, HID], f32, tag="main", name="psagg")
            first = True
            for (coff, ch) in blk_chunks[k]:
                gt = gp.tile([128, ch + 1, HID], f32, tag="gt", name="gt")
                qctr[0] ^= 1
                gi = nc.gpsimd.dma_gather(
                    gt[:], tab[FBASE:, :], idx_sb[:, coff:coff + (ch + 1) * 8],
                    (ch + 1) * 128, kreg[(ch + 1) * 128], HID, single_packet=False,
                    queue_num=qctr[0])
                add_dep_helper(gi.ins, cc.ins, reason="gather after AG")
                for r in range(ch):
                    nc.tensor.matmul(out=ps[:], lhsT=ident[:], rhs=gt[:, r, :],
                                     start=first, stop=False)
                    first = False
            nc.tensor.matmul(out=ps[:], lhsT=ident[:],
                             rhs=own_tile[:, k * 128:(k + 1) * 128],
                             start=first, stop=True)
            return ps

        # ---------------- Phase B: layer-1 aggregation -> g ----------------
        wr2 = []
        with nc.named_scope("agg1"):
            for k in range(BPC):
                ps = aggregate(k, tab1, hs_own, cc1)
                t1 = sp_.tile([128, HID], f32, tag="t1", name="t1")
                nc.scalar.activation(out=t1[:], in_=ps[:],
                                     func=mybir.ActivationFunctionType.Copy,
                                     scale=dinv[:, k:k + 1])
                t2 = sp_.tile([128, HID], f32, tag="t2", name="t2")
                nc.vector.tensor_add(out=t2[:], in0=t1[:], in1=bias1[:])
                nc.scalar.activation(out=g_own[:, k * 128:(k + 1) * 128], in_=t2[:],
                                     func=mybir.ActivationFunctionType.Relu,
                                     scale=dinv[:, k:k + 1])
                wr2.append(nc.sync.dma_start(out=ag2_in[k * 128:(k + 1) * 128, :],
                                             in_=g_own[:, k * 128:(k + 1) * 128]))

        with nc.named_scope("ag2"):
            cc2 = nc.gpsimd.collective_compute(
                "AllGather", mybir.AluOpType.bypass,
                replica_groups=[list(range(C))],
                ins=[ag2_in.ap().opt()], outs=[tab2.ap().opt()],
            )
            for w in wr2:
                add_dep_helper(cc2.ins, w.ins, reason="AG2 after g writes")

        # ---------------- Phase C: layer-2 aggregation + GEMM2 -> z ----------------
        wr3 = []
        with nc.named_scope("agg2"):
            for k in range(BPC):
                ps = aggregate(k, tab2, g_own, cc2)
                q = sp_.tile([128, HID], f32, tag="q", name="q")
                nc.scalar.activation(out=q[:], in_=ps[:],
                                     func=mybir.ActivationFunctionType.Copy,
                                     scale=dinv[:, k:k + 1])
                qt_ps = pz.tile([128, HID], f32, tag="qt", name="qtps")
                nc.tensor.transpose(out=qt_ps[:], in_=q[:], identity=ident[:])
                qt = sp_.tile([128, HID], f32, tag="qt_sb", name="qtsb")
                nc.vector.tensor_copy(out=qt[:], in_=qt_ps[:])
                zps = pz.tile([128, OUT], f32, tag="z", name="zps")
                nc.tensor.matmul(out=zps[:], lhsT=qt[:], rhs=W2_sb[:], start=True, stop=True)
                z = sp_.tile([128, OUT], f32, tag="zsb", name="zsb")
                nc.vector.tensor_add(out=z[:], in0=zps[:], in1=bias2[:])
                wr3.append(nc.sync.dma_start(out=ag3_in[k * 128:(k + 1) * 128, :], in_=z[:]))

        with nc.named_scope("ag3"):
            cc3 = nc.gpsimd.collective_compute(
                "AllGather", mybir.AluOpType.bypass,
                replica_groups=[list(range(C))],
                ins=[ag3_in.ap().opt()], outs=[ztab.ap().opt()],
            )
            for w in wr3:
                add_dep_helper(cc3.ins, w.ins, reason="AG3 after z writes")

        # ---------------- Phase D: decode ----------------
        with nc.named_scope("decode"):
            out_sb = own.tile([128, ndec_cols], f32)
            col = 0
            for i, (r0, ch) in enumerate(dec_chunks):
                za = dgp.tile([128, ch + 1, OUT], f32, tag="za", name="za")
                ga = nc.gpsimd.dma_gather(
                    za[:], ztab[FBASE:, :], idx_sb[:, dec_acoffs[i]:dec_acoffs[i] + (ch + 1) * 8],
                    (ch + 1) * 128, kreg[(ch + 1) * 128], OUT, single_packet=False,
                    queue_num=0)
                add_dep_helper(ga.ins, cc3.ins, reason="decode a after AG3")
                zb = dgp.tile([128, ch + 1, OUT], f32, tag="zb", name="zb")
                gb = nc.gpsimd.dma_gather(
                    zb[:], ztab[FBASE:, :], idx_sb[:, dec_bcoffs[i]:dec_bcoffs[i] + (ch + 1) * 8],
                    (ch + 1) * 128, kreg[(ch + 1) * 128], OUT, single_packet=False,
                    queue_num=1)
                add_dep_helper(gb.ins, cc3.ins, reason="decode b after AG3")
                prod = sp_.tile([128, ch * OUT], f32, tag="prod", name="prod")
                nc.vector.tensor_mul(out=prod[:].rearrange("p (c o) -> p c o", o=OUT),
                                     in0=za[:, :ch, :], in1=zb[:, :ch, :])
                nc.vector.reduce_sum(out=out_sb[:, col:col + ch],
                                     in_=prod[:].rearrange("p (c o) -> p c o", o=OUT),
                                     axis=mybir.AxisListType.X)
                col += ch
            nc.sync.dma_start(out=out_d[:, :], in_=out_sb[:])

    lower_extended_insts(nc)
    _fix_sync_waits(nc)
    return nc


def kernel(x, W1, b1, W2, b2, edge_index, edge_label_index):
    x = np.asarray(x, dtype=np.float32)
    W1 = np.asarray(W1, dtype=np.float32)
    b1 = np.asarray(b1, dtype=np.float32)
    W2 = np.asarray(W2, dtype=np.float32)
    b2 = np.asarray(b2, dtype=np.float32)
    prep = _prepare(np.asarray(edge_index), np.asarray(edge_label_index))
    nc = _build(prep)

    xp = np.zeros((NP, IN), dtype=np.float32)
    rm = prep["real_mask"]
    xp[rm] = x[prep["final_perm"][rm]]

    in_maps = []
    for c in range(C):
        in_maps.append({
            "xT": np.ascontiguousarray(xp[c * NPC:(c + 1) * NPC].T),
            "W1": W1, "b1": b1.reshape(1, HID),
            "W2": W2, "b2": b2.reshape(1, OUT),
            "degp": prep["degp_core"][c],
            "idx16": prep["idx16"][c],
        })
    res = run_bass_kernel_spmd(nc, in_maps, core_ids=list(range(C)))

    LPC = L // C
    out = np.empty(L, dtype=np.float32)
    for c in range(C):
        o = res.results[c]["out"]          # [128, ncols]; label j at (j%128, j//128)
        j = np.arange(LPC)
        out[c * LPC:(c + 1) * LPC] = o[j % 128, j // 128]
    return out


# revision 12
# speedup vs baseline: 1.9189x; 1.0196x over previous
"""GCN (2-layer) + edge-dot decode on 8 TRN2 NeuronCores.

Math (per GCN layer, with dinv = rsqrt(indeg+1)):
    out[v] = dinv[v] * ( sum_{e: dst=v} hs[src_e] + hs[v] ) + b,  hs = dinv (.) (x @ W)
so no per-edge norm values are needed anywhere; all scaling is per-node.

Layer 2 is commuted past W2:  z = (dinv (.) (A_hat g)) @ W2 + b2,  g = dinv (.) relu(out1).

Device layout: nodes permuted (degree-sorted, core-striped so each core owns a
contiguous 6272-row slice). Aggregation via dma_gather row-gathers + TensorE
identity-matmul accumulation into PSUM; hs/g/z tables AllGathered between
phases. dma_gather indices are SIGNED int16, so every gather uses a frame
centered at row 32768 (idx = row - 32768 covers the whole 50176-row table);
the ucode drops a trailing run of negative indices, so each gather appends one
all-positive pad round pointing at a zero (pad-node) row.
"""

import sys
import numpy as np
from contextlib import ExitStack

sys.path.insert(0, "/opt/trn_rl_repo")

import concourse.bass as bass
import concourse.mybir as mybir
from concourse.bass_utils import run_bass_kernel_spmd
from concourse.tile import TileContext, add_dep_helper
from concourse.masks import make_identity
from concourse.library_config import mlp
from concourse.library_overlay import lower_extended_insts

N, E, L = 50000, 800000, 200000
IN, HID, OUT = 256, 128, 64
C = 8                      # cores
NP = 50176                 # padded node count = 392 blocks of 128
NPC = NP // C              # 6272 nodes per core
BPC = NPC // 128           # 49 blocks per core
FBASE = 32768              # gather frame base row (signed int16 centered)
PADIDX = NP - 1 - FBASE    # pad index -> row 50175 (a zero pad-node row), > 0
CH_MAX = 24                # max rounds per gather chunk (excl. appended pad round)
DEC_CHUNK = 16             # decode chunk rounds

CUSTOM_ISA_OPCODES = {"DMAGatherAnt", "DMAScatterAddAnt"}


def _fix_sync_waits(nc):
    """This container's walrus accepts at most one sync-wait per instruction
    and none on custom ISA ucode ops; hoist extras onto preceding drains."""
    f = nc.m.functions[0]
    for b in f.blocks:
        insts = b.instructions
        i = 0
        while i < len(insts):
            ins = insts[i]
            si = ins.sync_info
            nw = len(si.on_wait) if (si is not None and si.on_wait is not None) else 0
            keep = 0 if str(ins.opcode) in CUSTOM_ISA_OPCODES else 1
            if nw > keep:
                waits = list(si.on_wait)
                hoist, keepw = waits[: nw - keep], waits[nw - keep:]
                for j, w in enumerate(hoist):
                    d = mybir.InstEventSemaphore(name=f"{ins.name}-wsplit{j}")
                    d.engine = ins.engine
                    d.sync_info = mybir.SyncInfo(on_wait=[w], on_update=[])
                    insts.insert(i + j, d)
                si.on_wait = keepw
                i += len(hoist)
            i += 1


def _sortedpos(p):
    """final position -> position in the degree-sorted sequence."""
    core = p // NPC
    k = (p % NPC) // 128
    lane = p % 128
    return 128 * (8 * k + core) + lane


SPLIT_O = 3200             # per-core first-half rows (25 blocks)


def _rowmap(p):
    """final position -> gather-table row: [all cores' first halves][second halves]."""
    c = p // NPC
    o = p % NPC
    return np.where(o < SPLIT_O, c * SPLIT_O + o,
                    C * SPLIT_O + c * (NPC - SPLIT_O) + (o - SPLIT_O))


def _wrap_idx(flat):
    """[n] int16 -> [128, n//16] wrapped in 16 partitions, replicated x8."""
    n = flat.shape[0]
    arr = np.empty((16, n // 16), dtype=np.int16)
    arr[:, :] = flat.reshape(n // 16, 16).T
    return np.tile(arr, (8, 1))


def _prepare(edge_index, edge_label_index):
    src = np.asarray(edge_index[0], dtype=np.int64)
    dst = np.asarray(edge_index[1], dtype=np.int64)
    la = np.asarray(edge_label_index[0], dtype=np.int64)
    lb = np.asarray(edge_label_index[1], dtype=np.int64)

    deg = np.bincount(dst, minlength=N).astype(np.int64)
    assert np.array_equal(np.sort(_rowmap(np.arange(NP))), np.arange(NP))
    assert _rowmap(np.asarray(NP - 1)) == NP - 1

    # permutation: degree-sorted, core-striped; 176 zero pad nodes at the tail
    sorted_real = np.argsort(-deg, kind="stable")
    seq = np.full(NP, -1, dtype=np.int64)
    seq[:N] = sorted_real
    final_perm = seq[_sortedpos(np.arange(NP))]   # final position -> orig (-1 pad)
    real_mask = final_perm >= 0
    invpos = np.full(N, -1, dtype=np.int64)
    invpos[final_perm[real_mask]] = np.nonzero(real_mask)[0]
    assert final_perm[NP - 1] == -1

    ps = invpos[src]
    pd = invpos[dst]

    # per-node in-edge ranks (dst-major)
    order = np.argsort(pd, kind="stable")
    pd_s = pd[order]
    ps_s = ps[order]
    newgrp = np.empty(E, dtype=bool)
    newgrp[0] = True
    newgrp[1:] = pd_s[1:] != pd_s[:-1]
    gidx = np.nonzero(newgrp)[0]
    rank = np.arange(E) - gidx[np.cumsum(newgrp) - 1]

    lane = pd_s % 128
    core = pd_s // NPC
    slot = (pd_s % NPC) // 128

    nblocks = NP // 128
    KB = np.zeros(nblocks, dtype=np.int64)
    np.maximum.at(KB, pd_s // 128, rank + 1)
    Khat = np.zeros(BPC, dtype=np.int64)
    for k in range(BPC):
        Khat[k] = int(KB[[c * BPC + k for c in range(C)]].max())
    off = np.zeros(BPC + 1, dtype=np.int64)
    off[1:] = np.cumsum(Khat)

    idxT = np.full((C, off[-1], 128), PADIDX, dtype=np.int16)
    idxT[core, off[slot] + rank, lane] = (_rowmap(ps_s) - FBASE).astype(np.int16)

    # chunk schedule per block: rounds split into chunks of <= CH_MAX, each
    # gather = chunk rounds + 1 appended all-pad round (trailing positivity)
    chunks = []   # (k, r0, ch)
    for k in range(BPC):
        r = 0
        while r < int(Khat[k]):
            ch = min(CH_MAX, int(Khat[k]) - r)
            chunks.append((k, r, ch))
            r += ch

    # degp per core [128, BPC]
    degp = np.full(NP, 1e30, dtype=np.float32)
    degp[real_mask] = (deg[final_perm[real_mask]] + 1).astype(np.float32)
    degp_core = degp.reshape(C, BPC, 128).transpose(0, 2, 1).copy()

    # decode tables: natural label order per core, chunks of DEC_CHUNK rounds
    pa = invpos[la]
    pb = invpos[lb]
    LPC = L // C
    LROUNDS = (LPC + 127) // 128
    dec_chunks = []
    r = 0
    while r < LROUNDS:
        ch = min(DEC_CHUNK, LROUNDS - r)
        dec_chunks.append((r, ch))
        r += ch

    padrow = np.full(128, PADIDX, dtype=np.int16)
    idx16 = []
    for c in range(C):
        parts = []
        for (k, r0, ch) in chunks:
            flat = idxT[c, off[k] + r0: off[k] + r0 + ch].reshape(-1)
            parts.append(_wrap_idx(np.concatenate([flat, padrow])))
        fa = np.full(LROUNDS * 128, PADIDX, dtype=np.int16)
        fb = np.full(LROUNDS * 128, PADIDX, dtype=np.int16)
        fa[:LPC] = (_rowmap(pa[c * LPC:(c + 1) * LPC]) - FBASE).astype(np.int16)
        fb[:LPC] = (_rowmap(pb[c * LPC:(c + 1) * LPC]) - FBASE).astype(np.int16)
        for (r0, ch) in dec_chunks:
            parts.append(_wrap_idx(np.concatenate([fa[r0 * 128:(r0 + ch) * 128], padrow])))
        for (r0, ch) in dec_chunks:
            parts.append(_wrap_idx(np.concatenate([fb[r0 * 128:(r0 + ch) * 128], padrow])))
        idx16.append(np.ascontiguousarray(np.concatenate(parts, axis=1)))

    return dict(
        final_perm=final_perm, invpos=invpos, real_mask=real_mask,
        Khat=Khat, off=off, chunks=chunks, dec_chunks=dec_chunks,
        degp_core=degp_core, idx16=idx16,
    )


def _build(prep):
    chunks = prep["chunks"]
    dec_chunks = prep["dec_chunks"]
    TOTW = prep["idx16"][0].shape[1]
    ndec_cols = sum(ch for (_, ch) in dec_chunks)

    nc = bass.Bass(num_devices=C, dynamic_dma_scratch_size=32768, num_swdge_queues=2)
    f32 = mybir.dt.float32
    xT_d = nc.dram_tensor("xT", [IN, NPC], f32, kind="ExternalInput")
    W1_d = nc.dram_tensor("W1", [IN, HID], f32, kind="ExternalInput")
    b1_d = nc.dram_tensor("b1", [1, HID], f32, kind="ExternalInput")
    W2_d = nc.dram_tensor("W2", [HID, OUT], f32, kind="ExternalInput")
    b2_d = nc.dram_tensor("b2", [1, OUT], f32, kind="ExternalInput")
    degp_d = nc.dram_tensor("degp", [128, BPC], f32, kind="ExternalInput")
    idx_d = nc.dram_tensor("idx16", [128, TOTW], mybir.dt.int16, kind="ExternalInput")
    out_d = nc.dram_tensor("out", [128, ndec_cols], f32, kind="ExternalOutput")

    ag1_in = nc.dram_tensor("ag1_in", [NPC, HID], f32)
    tab1 = nc.dram_tensor("tab1", [NP, HID], f32)
    ag2_in = nc.dram_tensor("ag2_in", [NPC, HID], f32)
    tab2 = nc.dram_tensor("tab2", [NP, HID], f32)
    ag3_in = nc.dram_tensor("ag3_in", [NPC, OUT], f32)
    ztab = nc.dram_tensor("ztab", [NP, OUT], f32)

    with TileContext(nc) as tc, ExitStack() as ctx:
        const = ctx.enter_context(tc.tile_pool(name="const", bufs=1))
        own = ctx.enter_context(tc.tile_pool(name="own", bufs=1))
        lp = ctx.enter_context(tc.tile_pool(name="lhsT", bufs=4))
        gp = ctx.enter_context(tc.tile_pool(name="gath", bufs=4))
        dgp = ctx.enter_context(tc.tile_pool(name="dgath", bufs=3))
        pp = ctx.enter_context(tc.tile_pool(name="psA", bufs=3, space="PSUM"))
        pz = ctx.enter_context(tc.tile_pool(name="psZ", bufs=2, space="PSUM"))
        sp_ = ctx.enter_context(tc.tile_pool(name="stage", bufs=4))

        ll = nc.gpsimd.load_library(mlp)

        ident = const.tile([128, 128], f32)
        make_identity(nc, ident[:])

        idx_sb = const.tile([128, TOTW], mybir.dt.int16)
        idma = nc.sync.dma_start(out=idx_sb[:], in_=idx_d[:, :])
        add_dep_helper(idma.ins, ll.ins, reason="idx after lib load")

        kvals = sorted({(ch + 1) * 128 for (_, _, ch) in chunks}
                       | {(ch + 1) * 128 for (_, ch) in dec_chunks})
        kreg = {}
        for v in kvals:
            r = ctx.enter_context(nc.gpsimd.register(f"nidx{v}"))
            nc.gpsimd.reg_mov(r, v)
            kreg[v] = r

        W1_sb = []
        for i in range(2):
            w1t = const.tile([128, HID], f32, tag=f"w1_{i}", name=f"w1_{i}")
            nc.sync.dma_start(out=w1t[:], in_=W1_d[i * 128:(i + 1) * 128, :])
            W1_sb.append(w1t)
        W2_sb = const.tile([128, OUT], f32)
        nc.sync.dma_start(out=W2_sb[:], in_=W2_d[:, :])

        ones_row = const.tile([1, 128], f32)
        nc.vector.memset(ones_row[:], 1.0)
        b1_row = const.tile([1, HID], f32)
        nc.sync.dma_start(out=b1_row[:], in_=b1_d[:, :])
        b2_row = const.tile([1, OUT], f32)
        nc.sync.dma_start(out=b2_row[:], in_=b2_d[:, :])
        bias1 = const.tile([128, HID], f32)
        bps = pz.tile([128, HID], f32, tag="qt")
        nc.tensor.matmul(out=bps[:], lhsT=ones_row[:], rhs=b1_row[:], start=True, stop=True)
        nc.scalar.activation(out=bias1[:], in_=bps[:], func=mybir.ActivationFunctionType.Copy)
        bias2 = const.tile([128, OUT], f32)
        bps2 = pz.tile([128, OUT], f32, tag="qt")
        nc.tensor.matmul(out=bps2[:], lhsT=ones_row[:], rhs=b2_row[:], start=True, stop=True)
        nc.scalar.activation(out=bias2[:], in_=bps2[:], func=mybir.ActivationFunctionType.Copy)

        degp_sb = const.tile([128, BPC], f32)
        nc.sync.dma_start(out=degp_sb[:], in_=degp_d[:, :])
        rec = const.tile([128, BPC], f32)
        nc.vector.reciprocal(out=rec[:], in_=degp_sb[:])
        dinv = const.tile([128, BPC], f32)
        nc.scalar.activation(out=dinv[:], in_=rec[:], func=mybir.ActivationFunctionType.Sqrt)

        hs_own = own.tile([128, NPC], f32)
        g_own = own.tile([128, NPC], f32)

        # chunk -> idx column offsets
        blk_chunks = [[] for _ in range(BPC)]
        co = 0
        for (k, r0, ch) in chunks:
            blk_chunks[k].append((co, ch))
            co += (ch + 1) * 8
        dec_acoffs = []
        for (r0, ch) in dec_chunks:
            dec_acoffs.append(co)
            co += (ch + 1) * 8
        dec_bcoffs = []
        for (r0, ch) in dec_chunks:
            dec_bcoffs.append(co)
            co += (ch + 1) * 8
        assert co == TOTW


        HA = SPLIT_O                    # 3200 rows -> tab rows [0, C*HA)
        HB = NPC - SPLIT_O              # 3072 rows -> tab rows [C*HA, NP)

        def split_allgather(ag_in, tab, wrs, scope):
            ccs = []
            with nc.named_scope(scope):
                cca = nc.gpsimd.collective_compute(
                    "AllGather", mybir.AluOpType.bypass,
                    replica_groups=[list(range(C))],
                    ins=[ag_in[0:HA, :].opt()], outs=[tab[0:C * HA, :].opt()],
                )
                for w in wrs[:HA // 128]:
                    add_dep_helper(cca.ins, w.ins, reason=f"{scope} a")
                ccb = nc.gpsimd.collective_compute(
                    "AllGather", mybir.AluOpType.bypass,
                    replica_groups=[list(range(C))],
                    ins=[ag_in[HA:NPC, :].opt()], outs=[tab[C * HA:NP, :].opt()],
                )
                for w in wrs[HA // 128:]:
                    add_dep_helper(ccb.ins, w.ins, reason=f"{scope} b")
                ccs = [cca, ccb]
            return ccs

        # ---------------- Phase A: GEMM1 -> hs1 ----------------
        wr1 = []
        with nc.named_scope("gemm1"):
            for k in range(BPC):
                ps = pp.tile([128, HID], f32, tag="main", name="psg")
                for i in range(2):
                    lt = lp.tile([128, 128], f32, tag="lhsT", name="lt")
                    nc.sync.dma_start(out=lt[:], in_=xT_d[i * 128:(i + 1) * 128, k * 128:(k + 1) * 128])
                    nc.tensor.matmul(out=ps[:], lhsT=lt[:], rhs=W1_sb[i][:],
                                     start=(i == 0), stop=(i == 1))
                nc.scalar.activation(out=hs_own[:, k * 128:(k + 1) * 128], in_=ps[:],
                                     func=mybir.ActivationFunctionType.Copy,
                                     scale=dinv[:, k:k + 1])
                wr1.append(nc.sync.dma_start(out=ag1_in[k * 128:(k + 1) * 128, :],
                                             in_=hs_own[:, k * 128:(k + 1) * 128]))

        cc1s = split_allgather(ag1_in, tab1, wr1, "ag1")

        qctr = [0]

        def aggregate(k, tab, own_tile, ccs):
            ps = pp.tile([128, HID], f32, tag="main", name="psagg")
            first = True
            for (coff, ch) in blk_chunks[k]:
                gt = gp.tile([128, ch + 1, HID], f32, tag="gt", name="gt")
                qctr[0] ^= 1
                gi = nc.gpsimd.dma_gather(
                    gt[:], tab[FBASE:, :], idx_sb[:, coff:coff + (ch + 1) * 8],
                    (ch + 1) * 128, kreg[(ch + 1) * 128], HID, single_packet=False,
                    queue_num=qctr[0])
                for cc in ccs:
                    add_dep_helper(gi.ins, cc.ins, reason="gather after AG")
                for r in range(ch):
                    nc.tensor.matmul(out=ps[:], lhsT=ident[:], rhs=gt[:, r, :],
                                     start=first, stop=False)
                    first = False
            nc.tensor.matmul(out=ps[:], lhsT=ident[:],
                             rhs=own_tile[:, k * 128:(k + 1) * 128],
                             start=first, stop=True)
            return ps

        # ---------------- Phase B: layer-1 aggregation -> g ----------------
        wr2 = []
        with nc.named_scope("agg1"):
            for k in range(BPC):
                ps = aggregate(k, tab1, hs_own, cc1s)
                t1 = sp_.tile([128, HID], f32, tag="t1", name="t1")
                nc.scalar.activation(out=t1[:], in_=ps[:],
                                     func=mybir.ActivationFunctionType.Copy,
                                     scale=dinv[:, k:k + 1])
                t2 = sp_.tile([128, HID], f32, tag="t2", name="t2")
                nc.vector.tensor_add(out=t2[:], in0=t1[:], in1=bias1[:])
                nc.scalar.activation(out=g_own[:, k * 128:(k + 1) * 128], in_=t2[:],
                                     func=mybir.ActivationFunctionType.Relu,
                                     scale=dinv[:, k:k + 1])
                wr2.append(nc.sync.dma_start(out=ag2_in[k * 128:(k + 1) * 128, :],
                                             in_=g_own[:, k * 128:(k + 1) * 128]))

        cc2s = split_allgather(ag2_in, tab2, wr2, "ag2")

        # ---------------- Phase C: layer-2 aggregation + GEMM2 -> z ----------------
        wr3 = []
        with nc.named_scope("agg2"):
            for k in range(BPC):
                ps = aggregate(k, tab2, g_own, cc2s)
                q = sp_.tile([128, HID], f32, tag="q", name="q")
                nc.scalar.activation(out=q[:], in_=ps[:],
                                     func=mybir.ActivationFunctionType.Copy,
                                     scale=dinv[:, k:k + 1])
                qt_ps = pz.tile([128, HID], f32, tag="qt", name="qtps")
                nc.tensor.transpose(out=qt_ps[:], in_=q[:], identity=ident[:])
                qt = sp_.tile([128, HID], f32, tag="qt_sb", name="qtsb")
                nc.vector.tensor_copy(out=qt[:], in_=qt_ps[:])
                zps = pz.tile([128, OUT], f32, tag="z", name="zps")
                nc.tensor.matmul(out=zps[:], lhsT=qt[:], rhs=W2_sb[:], start=True, stop=True)
                z = sp_.tile([128, OUT], f32, tag="zsb", name="zsb")
                nc.vector.tensor_add(out=z[:], in0=zps[:], in1=bias2[:])
                wr3.append(nc.sync.dma_start(out=ag3_in[k * 128:(k + 1) * 128, :], in_=z[:]))

        cc3s = split_allgather(ag3_in, ztab, wr3, "ag3")

        # ---------------- Phase D: decode ----------------
        with nc.named_scope("decode"):
            out_sb = own.tile([128, ndec_cols], f32)
            col = 0
            for i, (r0, ch) in enumerate(dec_chunks):
                za = dgp.tile([128, ch + 1, OUT], f32, tag="za", name="za")
                ga = nc.gpsimd.dma_gather(
                    za[:], ztab[FBASE:, :], idx_sb[:, dec_acoffs[i]:dec_acoffs[i] + (ch + 1) * 8],
                    (ch + 1) * 128, kreg[(ch + 1) * 128], OUT, single_packet=False,
                    queue_num=0)
                for cc in cc3s:
                    add_dep_helper(ga.ins, cc.ins, reason="decode a after AG3")
                zb = dgp.tile([128, ch + 1, OUT], f32, tag="zb", name="zb")
                gb = nc.gpsimd.dma_gather(
                    zb[:], ztab[FBASE:, :], idx_sb[:, dec_bcoffs[i]:dec_bcoffs[i] + (ch + 1) * 8],
                    (ch + 1) * 128, kreg[(ch + 1) * 128], OUT, single_packet=False,
                    queue_num=1)
                for cc in cc3s:
                    add_dep_helper(gb.ins, cc.ins, reason="decode b after AG3")
                prod = sp_.tile([128, ch * OUT], f32, tag="prod", name="prod")
                nc.vector.tensor_mul(out=prod[:].rearrange("p (c o) -> p c o", o=OUT),
                                     in0=za[:, :ch, :], in1=zb[:, :ch, :])
                nc.vector.reduce_sum(out=out_sb[:, col:col + ch],
                                     in_=prod[:].rearrange("p (c o) -> p c o", o=OUT),
                                     axis=mybir.AxisListType.X)
                col += ch
            nc.sync.dma_start(out=out_d[:, :], in_=out_sb[:])

    lower_extended_insts(nc)
    _fix_sync_waits(nc)
    return nc


def kernel(x, W1, b1, W2, b2, edge_index, edge_label_index):
    x = np.asarray(x, dtype=np.float32)
    W1 = np.asarray(W1, dtype=np.float32)
    b1 = np.asarray(b1, dtype=np.float32)
    W2 = np.asarray(W2, dtype=np.float32)
    b2 = np.asarray(b2, dtype=np.float32)
    prep = _prepare(np.asarray(edge_index), np.asarray(edge_label_index))
    nc = _build(prep)

    xp = np.zeros((NP, IN), dtype=np.float32)
    rm = prep["real_mask"]
    xp[rm] = x[prep["final_perm"][rm]]

    in_maps = []
    for c in range(C):
        in_maps.append({
            "xT": np.ascontiguousarray(xp[c * NPC:(c + 1) * NPC].T),
            "W1": W1, "b1": b1.reshape(1, HID),
            "W2": W2, "b2": b2.reshape(1, OUT),
            "degp": prep["degp_core"][c],
            "idx16": prep["idx16"][c],
        })
    res = run_bass_kernel_spmd(nc, in_maps, core_ids=list(range(C)))

    LPC = L // C
    out = np.empty(L, dtype=np.float32)
    for c in range(C):
        o = res.results[c]["out"]          # [128, ncols]; label j at (j%128, j//128)
        j = np.arange(LPC)
        out[c * LPC:(c + 1) * LPC] = o[j % 128, j // 128]
    return out


# revision 13
# speedup vs baseline: 1.9381x; 1.0100x over previous
"""GCN (2-layer) + edge-dot decode on 8 TRN2 NeuronCores.

Math (per GCN layer, with dinv = rsqrt(indeg+1)):
    out[v] = dinv[v] * ( sum_{e: dst=v} hs[src_e] + hs[v] ) + b,  hs = dinv (.) (x @ W)
so no per-edge norm values are needed anywhere; all scaling is per-node.

Layer 2 is commuted past W2:  z = (dinv (.) (A_hat g)) @ W2 + b2,  g = dinv (.) relu(out1).

Device layout: nodes permuted (degree-sorted, core-striped so each core owns a
contiguous 6272-row slice). Aggregation via dma_gather row-gathers + TensorE
identity-matmul accumulation into PSUM; hs/g/z tables AllGathered between
phases. dma_gather indices are SIGNED int16, so every gather uses a frame
centered at row 32768 (idx = row - 32768 covers the whole 50176-row table);
the ucode drops a trailing run of negative indices, so each gather appends one
all-positive pad round pointing at a zero (pad-node) row.
"""

import sys
import numpy as np
from contextlib import ExitStack

sys.path.insert(0, "/opt/trn_rl_repo")

import concourse.bass as bass
import concourse.mybir as mybir
from concourse.bass_utils import run_bass_kernel_spmd
from concourse.tile import TileContext, add_dep_helper
from concourse.masks import make_identity
from concourse.library_config import mlp
from concourse.library_overlay import lower_extended_insts

N, E, L = 50000, 800000, 200000
IN, HID, OUT = 256, 128, 64
C = 8                      # cores
NP = 50176                 # padded node count = 392 blocks of 128
NPC = NP // C              # 6272 nodes per core
BPC = NPC // 128           # 49 blocks per core
FBASE = 32768              # gather frame base row (signed int16 centered)
PADIDX = NP - 1 - FBASE    # pad index -> row 50175 (a zero pad-node row), > 0
CH_MAX = 24                # max rounds per gather chunk (excl. appended pad round)
DEC_CHUNK = 16             # decode chunk rounds

CUSTOM_ISA_OPCODES = {"DMAGatherAnt", "DMAScatterAddAnt"}


def _fix_sync_waits(nc):
    """This container's walrus accepts at most one sync-wait per instruction
    and none on custom ISA ucode ops; hoist extras onto preceding drains."""
    f = nc.m.functions[0]
    for b in f.blocks:
        insts = b.instructions
        i = 0
        while i < len(insts):
            ins = insts[i]
            si = ins.sync_info
            nw = len(si.on_wait) if (si is not None and si.on_wait is not None) else 0
            keep = 0 if str(ins.opcode) in CUSTOM_ISA_OPCODES else 1
            if nw > keep:
                waits = list(si.on_wait)
                hoist, keepw = waits[: nw - keep], waits[nw - keep:]
                for j, w in enumerate(hoist):
                    d = mybir.InstEventSemaphore(name=f"{ins.name}-wsplit{j}")
                    d.engine = ins.engine
                    d.sync_info = mybir.SyncInfo(on_wait=[w], on_update=[])
                    insts.insert(i + j, d)
                si.on_wait = keepw
                i += len(hoist)
            i += 1


def _sortedpos(p):
    """final position -> position in the degree-sorted sequence."""
    core = p // NPC
    k = (p % NPC) // 128
    lane = p % 128
    return 128 * (8 * k + core) + lane


SPLIT_O = 3200             # per-core first-half rows (25 blocks)


def _rowmap(p):
    """final position -> gather-table row: [all cores' first halves][second halves]."""
    c = p // NPC
    o = p % NPC
    return np.where(o < SPLIT_O, c * SPLIT_O + o,
                    C * SPLIT_O + c * (NPC - SPLIT_O) + (o - SPLIT_O))


def _wrap_idx(flat):
    """[n] int16 -> [128, n//16] wrapped in 16 partitions, replicated x8."""
    n = flat.shape[0]
    arr = np.empty((16, n // 16), dtype=np.int16)
    arr[:, :] = flat.reshape(n // 16, 16).T
    return np.tile(arr, (8, 1))


def _prepare(edge_index, edge_label_index):
    src = np.asarray(edge_index[0], dtype=np.int64)
    dst = np.asarray(edge_index[1], dtype=np.int64)
    la = np.asarray(edge_label_index[0], dtype=np.int64)
    lb = np.asarray(edge_label_index[1], dtype=np.int64)

    deg = np.bincount(dst, minlength=N).astype(np.int64)
    assert np.array_equal(np.sort(_rowmap(np.arange(NP))), np.arange(NP))
    assert _rowmap(np.asarray(NP - 1)) == NP - 1

    # permutation: degree-sorted, core-striped; 176 zero pad nodes at the tail
    sorted_real = np.argsort(-deg, kind="stable")
    seq = np.full(NP, -1, dtype=np.int64)
    seq[:N] = sorted_real
    final_perm = seq[_sortedpos(np.arange(NP))]   # final position -> orig (-1 pad)
    real_mask = final_perm >= 0
    invpos = np.full(N, -1, dtype=np.int64)
    invpos[final_perm[real_mask]] = np.nonzero(real_mask)[0]
    assert final_perm[NP - 1] == -1

    ps = invpos[src]
    pd = invpos[dst]

    # per-node in-edge ranks (dst-major)
    order = np.argsort(pd, kind="stable")
    pd_s = pd[order]
    ps_s = ps[order]
    newgrp = np.empty(E, dtype=bool)
    newgrp[0] = True
    newgrp[1:] = pd_s[1:] != pd_s[:-1]
    gidx = np.nonzero(newgrp)[0]
    rank = np.arange(E) - gidx[np.cumsum(newgrp) - 1]

    lane = pd_s % 128
    core = pd_s // NPC
    slot = (pd_s % NPC) // 128

    nblocks = NP // 128
    KB = np.zeros(nblocks, dtype=np.int64)
    np.maximum.at(KB, pd_s // 128, rank + 1)
    Khat = np.zeros(BPC, dtype=np.int64)
    for k in range(BPC):
        Khat[k] = int(KB[[c * BPC + k for c in range(C)]].max())
    off = np.zeros(BPC + 1, dtype=np.int64)
    off[1:] = np.cumsum(Khat)

    idxT = np.full((C, off[-1], 128), PADIDX, dtype=np.int16)
    idxT[core, off[slot] + rank, lane] = (_rowmap(ps_s) - FBASE).astype(np.int16)

    # chunk schedule per block: rounds split into chunks of <= CH_MAX, each
    # gather = chunk rounds + 1 appended all-pad round (trailing positivity)
    chunks = []   # (k, r0, ch)
    for k in range(BPC):
        r = 0
        while r < int(Khat[k]):
            ch = min(CH_MAX, int(Khat[k]) - r)
            chunks.append((k, r, ch))
            r += ch

    # degp per core [128, BPC]
    degp = np.full(NP, 1e30, dtype=np.float32)
    degp[real_mask] = (deg[final_perm[real_mask]] + 1).astype(np.float32)
    degp_core = degp.reshape(C, BPC, 128).transpose(0, 2, 1).copy()

    # decode tables: natural label order per core, chunks of DEC_CHUNK rounds
    pa = invpos[la]
    pb = invpos[lb]
    LPC = L // C
    LROUNDS = (LPC + 127) // 128
    dec_chunks = []
    r = 0
    while r < LROUNDS:
        ch = min(DEC_CHUNK, LROUNDS - r)
        dec_chunks.append((r, ch))
        r += ch

    padrow = np.full(128, PADIDX, dtype=np.int16)
    idx16 = []
    for c in range(C):
        parts = []
        for (k, r0, ch) in chunks:
            flat = idxT[c, off[k] + r0: off[k] + r0 + ch].reshape(-1)
            parts.append(_wrap_idx(np.concatenate([flat, padrow])))
        fa = np.full(LROUNDS * 128, PADIDX, dtype=np.int16)
        fb = np.full(LROUNDS * 128, PADIDX, dtype=np.int16)
        fa[:LPC] = (_rowmap(pa[c * LPC:(c + 1) * LPC]) - FBASE).astype(np.int16)
        fb[:LPC] = (_rowmap(pb[c * LPC:(c + 1) * LPC]) - FBASE).astype(np.int16)
        for (r0, ch) in dec_chunks:
            parts.append(_wrap_idx(np.concatenate([fa[r0 * 128:(r0 + ch) * 128], padrow])))
        for (r0, ch) in dec_chunks:
            parts.append(_wrap_idx(np.concatenate([fb[r0 * 128:(r0 + ch) * 128], padrow])))
        idx16.append(np.ascontiguousarray(np.concatenate(parts, axis=1)))

    return dict(
        final_perm=final_perm, invpos=invpos, real_mask=real_mask,
        Khat=Khat, off=off, chunks=chunks, dec_chunks=dec_chunks,
        degp_core=degp_core, idx16=idx16,
    )


def _build(prep):
    chunks = prep["chunks"]
    dec_chunks = prep["dec_chunks"]
    TOTW = prep["idx16"][0].shape[1]
    ndec_cols = sum(ch for (_, ch) in dec_chunks)

    nc = bass.Bass(num_devices=C, dynamic_dma_scratch_size=32768, num_swdge_queues=2)
    f32 = mybir.dt.float32
    xT_d = nc.dram_tensor("xT", [IN, NPC], f32, kind="ExternalInput")
    W1_d = nc.dram_tensor("W1", [IN, HID], f32, kind="ExternalInput")
    b1_d = nc.dram_tensor("b1", [1, HID], f32, kind="ExternalInput")
    W2_d = nc.dram_tensor("W2", [HID, OUT], f32, kind="ExternalInput")
    b2_d = nc.dram_tensor("b2", [1, OUT], f32, kind="ExternalInput")
    degp_d = nc.dram_tensor("degp", [128, BPC], f32, kind="ExternalInput")
    idx_d = nc.dram_tensor("idx16", [128, TOTW], mybir.dt.int16, kind="ExternalInput")
    out_d = nc.dram_tensor("out", [128, ndec_cols], f32, kind="ExternalOutput")

    ag1_in = nc.dram_tensor("ag1_in", [NPC, HID], f32)
    tab1 = nc.dram_tensor("tab1", [NP, HID], f32)
    ag2_in = nc.dram_tensor("ag2_in", [NPC, HID], f32)
    tab2 = nc.dram_tensor("tab2", [NP, HID], f32)
    ag3_in = nc.dram_tensor("ag3_in", [NPC, OUT], f32)
    ztab = nc.dram_tensor("ztab", [NP, OUT], f32)

    with TileContext(nc) as tc, ExitStack() as ctx:
        const = ctx.enter_context(tc.tile_pool(name="const", bufs=1))
        own = ctx.enter_context(tc.tile_pool(name="own", bufs=1))
        lp = ctx.enter_context(tc.tile_pool(name="lhsT", bufs=4))
        gp = ctx.enter_context(tc.tile_pool(name="gath", bufs=5))
        dgp = ctx.enter_context(tc.tile_pool(name="dgath", bufs=3))
        pp = ctx.enter_context(tc.tile_pool(name="psA", bufs=4, space="PSUM"))
        pz = ctx.enter_context(tc.tile_pool(name="psZ", bufs=2, space="PSUM"))
        sp_ = ctx.enter_context(tc.tile_pool(name="stage", bufs=4))

        ll = nc.gpsimd.load_library(mlp)

        ident = const.tile([128, 128], f32)
        make_identity(nc, ident[:])

        idx_sb = const.tile([128, TOTW], mybir.dt.int16)
        idma = nc.sync.dma_start(out=idx_sb[:], in_=idx_d[:, :])
        add_dep_helper(idma.ins, ll.ins, reason="idx after lib load")

        kvals = sorted({(ch + 1) * 128 for (_, _, ch) in chunks}
                       | {(ch + 1) * 128 for (_, ch) in dec_chunks})
        kreg = {}
        for v in kvals:
            r = ctx.enter_context(nc.gpsimd.register(f"nidx{v}"))
            nc.gpsimd.reg_mov(r, v)
            kreg[v] = r

        W1_sb = []
        for i in range(2):
            w1t = const.tile([128, HID], f32, tag=f"w1_{i}", name=f"w1_{i}")
            nc.sync.dma_start(out=w1t[:], in_=W1_d[i * 128:(i + 1) * 128, :])
            W1_sb.append(w1t)
        W2_sb = const.tile([128, OUT], f32)
        nc.sync.dma_start(out=W2_sb[:], in_=W2_d[:, :])

        ones_row = const.tile([1, 128], f32)
        nc.vector.memset(ones_row[:], 1.0)
        b1_row = const.tile([1, HID], f32)
        nc.sync.dma_start(out=b1_row[:], in_=b1_d[:, :])
        b2_row = const.tile([1, OUT], f32)
        nc.sync.dma_start(out=b2_row[:], in_=b2_d[:, :])
        bias1 = const.tile([128, HID], f32)
        bps = pz.tile([128, HID], f32, tag="qt")
        nc.tensor.matmul(out=bps[:], lhsT=ones_row[:], rhs=b1_row[:], start=True, stop=True)
        nc.scalar.activation(out=bias1[:], in_=bps[:], func=mybir.ActivationFunctionType.Copy)
        bias2 = const.tile([128, OUT], f32)
        bps2 = pz.tile([128, OUT], f32, tag="qt")
        nc.tensor.matmul(out=bps2[:], lhsT=ones_row[:], rhs=b2_row[:], start=True, stop=True)
        nc.scalar.activation(out=bias2[:], in_=bps2[:], func=mybir.ActivationFunctionType.Copy)

        degp_sb = const.tile([128, BPC], f32)
        nc.sync.dma_start(out=degp_sb[:], in_=degp_d[:, :])
        rec = const.tile([128, BPC], f32)
        nc.vector.reciprocal(out=rec[:], in_=degp_sb[:])
        dinv = const.tile([128, BPC], f32)
        nc.scalar.activation(out=dinv[:], in_=rec[:], func=mybir.ActivationFunctionType.Sqrt)

        hs_own = own.tile([128, NPC], f32)
        g_own = own.tile([128, NPC], f32)

        # chunk -> idx column offsets
        blk_chunks = [[] for _ in range(BPC)]
        co = 0
        for (k, r0, ch) in chunks:
            blk_chunks[k].append((co, ch))
            co += (ch + 1) * 8
        dec_acoffs = []
        for (r0, ch) in dec_chunks:
            dec_acoffs.append(co)
            co += (ch + 1) * 8
        dec_bcoffs = []
        for (r0, ch) in dec_chunks:
            dec_bcoffs.append(co)
            co += (ch + 1) * 8
        assert co == TOTW


        HA = SPLIT_O                    # 3200 rows -> tab rows [0, C*HA)
        HB = NPC - SPLIT_O              # 3072 rows -> tab rows [C*HA, NP)

        def split_allgather(ag_in, tab, wrs, scope):
            ccs = []
            with nc.named_scope(scope):
                cca = nc.gpsimd.collective_compute(
                    "AllGather", mybir.AluOpType.bypass,
                    replica_groups=[list(range(C))],
                    ins=[ag_in[0:HA, :].opt()], outs=[tab[0:C * HA, :].opt()],
                )
                for w in wrs[:HA // 128]:
                    add_dep_helper(cca.ins, w.ins, reason=f"{scope} a")
                ccb = nc.gpsimd.collective_compute(
                    "AllGather", mybir.AluOpType.bypass,
                    replica_groups=[list(range(C))],
                    ins=[ag_in[HA:NPC, :].opt()], outs=[tab[C * HA:NP, :].opt()],
                )
                for w in wrs[HA // 128:]:
                    add_dep_helper(ccb.ins, w.ins, reason=f"{scope} b")
                ccs = [cca, ccb]
            return ccs

        # ---------------- Phase A: GEMM1 -> hs1 ----------------
        wr1 = []
        with nc.named_scope("gemm1"):
            for k in range(BPC):
                ps = pp.tile([128, HID], f32, tag="main", name="psg")
                for i in range(2):
                    lt = lp.tile([128, 128], f32, tag="lhsT", name="lt")
                    nc.sync.dma_start(out=lt[:], in_=xT_d[i * 128:(i + 1) * 128, k * 128:(k + 1) * 128])
                    nc.tensor.matmul(out=ps[:], lhsT=lt[:], rhs=W1_sb[i][:],
                                     start=(i == 0), stop=(i == 1))
                nc.scalar.activation(out=hs_own[:, k * 128:(k + 1) * 128], in_=ps[:],
                                     func=mybir.ActivationFunctionType.Copy,
                                     scale=dinv[:, k:k + 1])
                wr1.append(nc.sync.dma_start(out=ag1_in[k * 128:(k + 1) * 128, :],
                                             in_=hs_own[:, k * 128:(k + 1) * 128]))

        cc1s = split_allgather(ag1_in, tab1, wr1, "ag1")

        qctr = [0]

        def aggregate(k, tab, own_tile, ccs):
            ps = pp.tile([128, HID], f32, tag="main", name="psagg")
            first = True
            for (coff, ch) in blk_chunks[k]:
                gt = gp.tile([128, ch + 1, HID], f32, tag="gt", name="gt")
                qctr[0] ^= 1
                gi = nc.gpsimd.dma_gather(
                    gt[:], tab[FBASE:, :], idx_sb[:, coff:coff + (ch + 1) * 8],
                    (ch + 1) * 128, kreg[(ch + 1) * 128], HID, single_packet=False,
                    queue_num=qctr[0])
                for cc in ccs:
                    add_dep_helper(gi.ins, cc.ins, reason="gather after AG")
                for r in range(ch):
                    nc.tensor.matmul(out=ps[:], lhsT=ident[:], rhs=gt[:, r, :],
                                     start=first, stop=False)
                    first = False
            nc.tensor.matmul(out=ps[:], lhsT=ident[:],
                             rhs=own_tile[:, k * 128:(k + 1) * 128],
                             start=first, stop=True)
            return ps

        # ---------------- Phase B: layer-1 aggregation -> g ----------------
        wr2 = []
        with nc.named_scope("agg1"):
            for k in range(BPC):
                ps = aggregate(k, tab1, hs_own, cc1s)
                t1 = sp_.tile([128, HID], f32, tag="t1", name="t1")
                nc.scalar.activation(out=t1[:], in_=ps[:],
                                     func=mybir.ActivationFunctionType.Copy,
                                     scale=dinv[:, k:k + 1])
                t2 = sp_.tile([128, HID], f32, tag="t2", name="t2")
                nc.vector.tensor_add(out=t2[:], in0=t1[:], in1=bias1[:])
                nc.scalar.activation(out=g_own[:, k * 128:(k + 1) * 128], in_=t2[:],
                                     func=mybir.ActivationFunctionType.Relu,
                                     scale=dinv[:, k:k + 1])
                wr2.append(nc.sync.dma_start(out=ag2_in[k * 128:(k + 1) * 128, :],
                                             in_=g_own[:, k * 128:(k + 1) * 128]))

        cc2s = split_allgather(ag2_in, tab2, wr2, "ag2")

        # ---------------- Phase C: layer-2 aggregation + GEMM2 -> z ----------------
        wr3 = []
        with nc.named_scope("agg2"):
            for k in range(BPC):
                ps = aggregate(k, tab2, g_own, cc2s)
                q = sp_.tile([128, HID], f32, tag="q", name="q")
                nc.scalar.activation(out=q[:], in_=ps[:],
                                     func=mybir.ActivationFunctionType.Copy,
                                     scale=dinv[:, k:k + 1])
                qt_ps = pz.tile([128, HID], f32, tag="qt", name="qtps")
                nc.tensor.transpose(out=qt_ps[:], in_=q[:], identity=ident[:])
                qt = sp_.tile([128, HID], f32, tag="qt_sb", name="qtsb")
                nc.vector.tensor_copy(out=qt[:], in_=qt_ps[:])
                zps = pz.tile([128, OUT], f32, tag="z", name="zps")
                nc.tensor.matmul(out=zps[:], lhsT=qt[:], rhs=W2_sb[:], start=True, stop=True)
                z = sp_.tile([128, OUT], f32, tag="zsb", name="zsb")
                nc.vector.tensor_add(out=z[:], in0=zps[:], in1=bias2[:])
                wr3.append(nc.sync.dma_start(out=ag3_in[k * 128:(k + 1) * 128, :], in_=z[:]))

        cc3s = split_allgather(ag3_in, ztab, wr3, "ag3")

        # ---------------- Phase D: decode ----------------
        with nc.named_scope("decode"):
            out_sb = own.tile([128, ndec_cols], f32)
            col = 0
            for i, (r0, ch) in enumerate(dec_chunks):
                za = dgp.tile([128, ch + 1, OUT], f32, tag="za", name="za")
                ga = nc.gpsimd.dma_gather(
                    za[:], ztab[FBASE:, :], idx_sb[:, dec_acoffs[i]:dec_acoffs[i] + (ch + 1) * 8],
                    (ch + 1) * 128, kreg[(ch + 1) * 128], OUT, single_packet=False,
                    queue_num=0)
                for cc in cc3s:
                    add_dep_helper(ga.ins, cc.ins, reason="decode a after AG3")
                zb = dgp.tile([128, ch + 1, OUT], f32, tag="zb", name="zb")
                gb = nc.gpsimd.dma_gather(
                    zb[:], ztab[FBASE:, :], idx_sb[:, dec_bcoffs[i]:dec_bcoffs[i] + (ch + 1) * 8],
                    (ch + 1) * 128, kreg[(ch + 1) * 128], OUT, single_packet=False,
                    queue_num=1)
                for cc in cc3s:
                    add_dep_helper(gb.ins, cc.ins, reason="decode b after AG3")
                prod = sp_.tile([128, ch * OUT], f32, tag="prod", name="prod")
                nc.vector.tensor_mul(out=prod[:].rearrange("p (c o) -> p c o", o=OUT),
                                     in0=za[:, :ch, :], in1=zb[:, :ch, :])
                nc.vector.reduce_sum(out=out_sb[:, col:col + ch],
                                     in_=prod[:].rearrange("p (c o) -> p c o", o=OUT),
                                     axis=mybir.AxisListType.X)
                col += ch
            nc.sync.dma_start(out=out_d[:, :], in_=out_sb[:])

    lower_extended_insts(nc)
    _fix_sync_waits(nc)
    return nc


def kernel(x, W1, b1, W2, b2, edge_index, edge_label_index):
    x = np.asarray(x, dtype=np.float32)
    W1 = np.asarray(W1, dtype=np.float32)
    b1 = np.asarray(b1, dtype=np.float32)
    W2 = np.asarray(W2, dtype=np.float32)
    b2 = np.asarray(b2, dtype=np.float32)
    prep = _prepare(np.asarray(edge_index), np.asarray(edge_label_index))
    nc = _build(prep)

    xp = np.zeros((NP, IN), dtype=np.float32)
    rm = prep["real_mask"]
    xp[rm] = x[prep["final_perm"][rm]]

    in_maps = []
    for c in range(C):
        in_maps.append({
            "xT": np.ascontiguousarray(xp[c * NPC:(c + 1) * NPC].T),
            "W1": W1, "b1": b1.reshape(1, HID),
            "W2": W2, "b2": b2.reshape(1, OUT),
            "degp": prep["degp_core"][c],
            "idx16": prep["idx16"][c],
        })
    res = run_bass_kernel_spmd(nc, in_maps, core_ids=list(range(C)))

    LPC = L // C
    out = np.empty(L, dtype=np.float32)
    for c in range(C):
        o = res.results[c]["out"]          # [128, ncols]; label j at (j%128, j//128)
        j = np.arange(LPC)
        out[c * LPC:(c + 1) * LPC] = o[j % 128, j // 128]
    return out
